# revision 1
# baseline (speedup 1.0000x reference)
"""NetVLAD (vq_codebook) Trainium2 Bass kernel, 8-way spatially sharded.

Math (verified vs reference to ~2e-7 rel):
  xn = x / ||x||_C per location; logits = conv_w @ xn; soft = softmax_K
  fold(unfold(soft) * top2keep) == soft * cnt, where cnt = 3x3 box-sum of the
  per-cluster top-2 indicator (border discrepancies are killed by the
  (min-dist-to-border)^4 mask). vlad = sa2 @ xn.T - rowsum(sa2) * centroids,
  then intra + global L2 norm.

Sharding: H=192 rows split 8 ways (24 rows/core + 1 halo row each side).
conv/softmax/top2/box-sum local per core; [K,C+1] partial VLAD sums
all-reduced across the 8 cores; final normalization redundantly on each core.
"""
import os
import sys

sys.path.insert(0, "/opt/trn_rl_repo")
os.environ.setdefault("MYCRO_LOCAL_CACHE", "1")

import numpy as np

C, H, W, K = 512, 192, 192, 64
M = 8                      # cores
RPC = H // M               # 24 rows per core
Ls = (RPC + 2) * W         # 4992 slab locations (incl. 1 halo row each side)
NT = Ls // 128             # 39 l-tiles
CT = C // 128              # 4 c-tiles
G = 257                    # odd guard -> v-pass offsets even (bf16 2x mode)
KBW = G + Ls + G           # 5506 keep-buffer width
XW = 8                     # xlc DMA batching (tiles per DMA)

TRACE = False              # set by test.py for profiling runs
_CACHE = {}


def _build_nc():
    import concourse.bass as bass
    import concourse.bass_isa as bass_isa
    import concourse.tile as tile
    from concourse import mybir

    f32 = mybir.dt.float32
    bf16 = mybir.dt.bfloat16
    AF = mybir.ActivationFunctionType
    OP = mybir.AluOpType
    AX = mybir.AxisListType

    nc = bass.Bass()
    xcl = nc.dram_tensor("xcl", [C, Ls], f32, kind="ExternalInput")
    xlcn = nc.dram_tensor("xlcn", [Ls, C + 1], f32, kind="ExternalInput")
    cwt = nc.dram_tensor("cwt", [C, K], f32, kind="ExternalInput")
    cent = nc.dram_tensor("cent", [K, C], f32, kind="ExternalInput")
    sc0 = nc.dram_tensor("sc0", [128, NT], f32, kind="ExternalInput")
    invn = nc.dram_tensor("invn", [128, NT], f32, kind="ExternalInput")
    identb = nc.dram_tensor("identb", [128, 128], bf16, kind="ExternalInput")
    identf = nc.dram_tensor("identf", [128, 128], f32, kind="ExternalInput")
    ones = nc.dram_tensor("ones", [128, 128], f32, kind="ExternalInput")
    y = nc.dram_tensor("y", [K, C + 1], f32, kind="ExternalOutput")

    with tile.TileContext(nc) as tc:
        with tc.tile_pool(name="big", bufs=1) as big:
            # persistent SBUF tensors
            expb = big.tile([128, NT * K], f32, tag="expb")
            tmpb = big.tile([128, NT * K], f32, tag="tmpb")   # also reused as w2
            keeplk = big.tile([128, NT * K], bf16, tag="keeplk")
            kb = big.tile([K, KBW], bf16, tag="kb")
            h3s = big.tile([K, KBW], bf16, tag="h3s")
            cntb = big.tile([K, Ls], bf16, tag="cntb")
            cwt_sb = big.tile([128, CT * K], f32, tag="cwt")
            cent_sb = big.tile([K, C], f32, tag="cent")
            id_sb = big.tile([128, 128], bf16, tag="ident")
            idf_sb = big.tile([128, 128], f32, tag="identf")
            logkl = big.tile([K, Ls], f32, tag="logkl")
            ones_sb = big.tile([128, 128], f32, tag="ones")
            sc_sb = big.tile([128, NT], f32, tag="sc0")
            invn_sb = big.tile([128, NT], f32, tag="invn")
            sume = big.tile([128, NT], f32, tag="sume")
            m1b = big.tile([128, NT], f32, tag="m1b")
            m2b = big.tile([128, NT], f32, tag="m2b")
            isum = big.tile([128, NT], f32, tag="isum")
            scc = big.tile([128, NT], f32, tag="scc")
            vl_sb = big.tile([K, C + 1], f32, tag="vl")
            scr = big.tile([128, 4], f32, tag="scr")

            # input DMAs
            nc.sync.dma_start(id_sb[:], identb[:])
            nc.sync.dma_start(idf_sb[:], identf[:])
            nc.sync.dma_start(ones_sb[:], ones[:])
            nc.sync.dma_start(sc_sb[:], sc0[:])
            nc.sync.dma_start(invn_sb[:], invn[:])
            nc.sync.dma_start(cent_sb[:], cent[:])
            nc.sync.dma_start(
                cwt_sb[:].rearrange("p (t k) -> p t k", k=K),
                cwt[:].rearrange("(t p) k -> p t k", p=128),
            )
            # zero the keep-buffer guards
            nc.vector.memset(kb[:, 0:G], 0.0)
            nc.vector.memset(kb[:, G + Ls:KBW], 0.0)
            # single-wait "touch" ops: each absorbs one DMA completion so no
            # downstream compute instruction needs two sync waits (codegen
            # allows one wait per compute-engine instruction)
            nc.scalar.copy(scr[:, 0:1], invn_sb[:, 0:1])
            nc.vector.tensor_copy(scr[:, 1:2], sc_sb[:, 0:1])

            # One persistent PSUM pool; reuse goes through tag rotation so each
            # PE instruction carries at most one sync wait (codegen limit).
            # Banks: plk 2 + plg 2 + pk 2 + pv0/pv1 2 = 8.
            with tc.tile_pool(name="pp", bufs=1, space="PSUM") as pp:
                pv0 = pp.tile([K, C], f32, tag="pv0", bufs=1)
                pv1 = pp.tile([K, 1], f32, tag="pv1", bufs=1)
                # warm-up burst: absorbs the cwt/ones DMA waits (1-wait codegen
                # limit) and keeps the PE HAM busy so phase 1 runs at 2.4 GHz
                dummy = pp.tile([128, K], f32, tag="plg", bufs=2)
                nc.tensor.matmul(dummy[0:64, 0:64], lhsT=cwt_sb[:, 0:64],
                                 rhs=cwt_sb[:, 0:64], start=True, stop=True)
                for _ in range(12):
                    dummy = pp.tile([128, K], f32, tag="plg", bufs=2)
                    nc.tensor.matmul(dummy[0:64, 0:64], lhsT=ones_sb[:, 0:64],
                                     rhs=ones_sb[:, 0:64], start=True, stop=True)
                # phase 1: logits matmuls + exp (scaled by inv_norm).
                # xcl lives in a scoped pool; its space is reused for the
                # xlcn stream afterwards (fresh addresses -> single-wait DMAs)
                with tc.tile_pool(name="xclp", bufs=1) as xclp:
                    xcl_sb = xclp.tile([128, CT * Ls], f32, tag="xcl")
                    xc3 = xcl[:].rearrange("(ct p) l -> p ct l", p=128)
                    xs3 = xcl_sb[:].rearrange("p (ct l) -> p ct l", l=Ls)
                    NB = Ls // 512          # 9.75 -> use 512-blocks + tail
                    DCH = 4                 # dma chunks (early phase-1 start)
                    csz = Ls // DCH         # 1248 columns per chunk, all c-tiles
                    for j in range(DCH):
                        nc.sync.dma_start(
                            xs3[:, :, j * csz:(j + 1) * csz],
                            xc3[:, :, j * csz:(j + 1) * csz],
                        )
                    # logits blocks in [K, L]: stationary conv_wT (64-col
                    # weight loads), x streams as the moving operand
                    nblk = (Ls + 511) // 512
                    touched = set()
                    for b in range(nblk):
                        w = min(512, Ls - b * 512)
                        for j in range((b * 512) // csz,
                                       (b * 512 + w - 1) // csz + 1):
                            if j not in touched:
                                touched.add(j)
                                dj = pp.tile([128, K], f32, tag="plg", bufs=2)
                                nc.tensor.matmul(
                                    dj[0:64, 0:64],
                                    lhsT=xcl_sb[:, j * csz:j * csz + 64],
                                    rhs=xcl_sb[:, j * csz:j * csz + 64],
                                    start=True, stop=True)
                        plk = pp.tile([K, 512], f32, tag="plk", bufs=2)
                        for ct in range(CT):
                            nc.tensor.matmul(
                                plk[:, 0:w],
                                lhsT=cwt_sb[:, ct * K:(ct + 1) * K],
                                rhs=xcl_sb[:, ct * Ls + b * 512:
                                           ct * Ls + b * 512 + w],
                                start=(ct == 0),
                                stop=(ct == CT - 1),
                            )
                        nc.scalar.copy(logkl[:, b * 512:b * 512 + w],
                                       plk[:, 0:w])
                    # transpose to [L-tile, K] and exp with per-location
                    # inv-norm scale; sumexp accumulates for free
                    for t in range(NT):
                        plg = pp.tile([128, K], f32, tag="plg", bufs=2)
                        nc.tensor.transpose(
                            plg[:], logkl[:, t * 128:(t + 1) * 128],
                            idf_sb[0:K, 0:K])
                        nc.scalar.activation(
                            expb[:, t * K:(t + 1) * K], plg[:], AF.Exp,
                            scale=invn_sb[:, t:t + 1],
                            accum_out=sume[:, t:t + 1],
                        )

                # phase 2: batched top-2 keep over the free axis
                e3 = expb[:].rearrange("p (t k) -> p t k", k=K)
                t3 = tmpb[:].rearrange("p (t k) -> p t k", k=K)
                k3 = keeplk[:].rearrange("p (t k) -> p t k", k=K)
                m1bc = m1b[:][:, :, None].broadcast_to([128, NT, K])
                m2bc = m2b[:][:, :, None].broadcast_to([128, NT, K])
                nc.vector.tensor_reduce(m1b[:], e3, axis=AX.X, op=OP.max)
                nc.vector.tensor_tensor(t3, e3, m1bc, op=OP.is_ge)
                nc.vector.scalar_tensor_tensor(
                    t3, t3, -10.0, e3, op0=OP.mult, op1=OP.add)
                nc.vector.tensor_reduce(m2b[:], t3, axis=AX.X, op=OP.max)
                nc.vector.tensor_tensor(k3, e3, m2bc, op=OP.is_ge)
                nc.vector.reciprocal(isum[:], sume[:])
                nc.vector.tensor_mul(scc[:], sc_sb[:], isum[:])

                # phase 3: transpose keep [L,K] -> [K,L] into guarded buffer
                for t in range(NT):
                    pk = pp.tile([K, 128], bf16, tag="pk", bufs=2)
                    nc.tensor.transpose(
                        pk[:], keeplk[:, t * K:(t + 1) * K], id_sb[:])
                    nc.scalar.copy(kb[:, G + t * 128: G + (t + 1) * 128], pk[:])

                # phase 4: separable 3x3 box-sum along flattened L
                # h3s[j] = kb[j] + kb[j+1] + kb[j+2]  (i.e. h[j+1], shifted)
                nc.vector.tensor_add(
                    h3s[:, 0:KBW - 2], kb[:, 0:KBW - 2], kb[:, 2:KBW])
                nc.vector.tensor_add(
                    h3s[:, 0:KBW - 2], h3s[:, 0:KBW - 2], kb[:, 1:KBW - 1])
                # cnt[l] = h[G+l-192] + h[G+l] + h[G+l+192], h[j] = h3s[j-1]
                nc.vector.tensor_add(
                    cntb[:], h3s[:, G - 193:G - 193 + Ls],
                    h3s[:, G + 191:G + 191 + Ls])
                nc.vector.tensor_add(
                    cntb[:], cntb[:], h3s[:, G - 1:G - 1 + Ls])

                # phase 5: transpose cnt back, fuse w2 = (cntT * scc) * exp
                w2 = tmpb
                for t in range(NT):
                    pc = pp.tile([128, K], bf16, tag="plg", bufs=2)
                    nc.tensor.transpose(
                        pc[:], cntb[:, t * 128:(t + 1) * 128], id_sb[:K, :K])
                    nc.vector.scalar_tensor_tensor(
                        w2[:, t * K:(t + 1) * K], pc[:], scc[:, t:t + 1],
                        expb[:, t * K:(t + 1) * K], op0=OP.mult, op1=OP.mult)

                # absorb the w2 DVE wait before the accumulation chain
                dummy2 = pp.tile([128, K], f32, tag="plg", bufs=2)
                nc.tensor.matmul(dummy2[0:64, 0:64], lhsT=w2[:, 0:64],
                                 rhs=w2[:, 0:64], start=True, stop=True)

                # phase 6: VLAD matmul, accumulate [K, C+1] over all l-tiles.
                # Each xlcn wave gets its own buffer (in space freed by xclp)
                # so stream DMAs carry a single sync wait.
                x3 = xlcn[:].rearrange("(a p) c -> p a c", p=128)
                with tc.tile_pool(name="xlc", bufs=1) as xlcp:
                    for w in range((NT + XW - 1) // XW):
                        n = min(XW, NT - w * XW)
                        xt = xlcp.tile([128, XW * (C + 1)], f32, tag=f"xt{w}")
                        nc.sync.dma_start(
                            xt[:, 0:n * (C + 1)].rearrange(
                                "p (a c) -> p a c", c=C + 1),
                            x3[:, w * XW:w * XW + n, :],
                        )
                        for i in range(n):
                            t = w * XW + i
                            lt = w2[:, t * K:(t + 1) * K]
                            nc.tensor.matmul(
                                pv0[:], lhsT=lt,
                                rhs=xt[:, i * (C + 1):i * (C + 1) + C],
                                start=(t == 0), stop=(t == NT - 1))
                            nc.tensor.matmul(
                                pv1[:], lhsT=lt,
                                rhs=xt[:, i * (C + 1) + C:(i + 1) * (C + 1)],
                                start=(t == 0), stop=(t == NT - 1))

                    # phase 7: write this core's [K, C+1] partial sums;
                    # host sums the 8 partials and applies centroid subtraction
                    # and the two L2 normalizations (0.03% of the FLOPs)
                    nc.scalar.copy(vl_sb[:, 0:C], pv0[:])
                    nc.scalar.copy(vl_sb[:, C:C + 1], pv1[:])
                    nc.sync.dma_start(y[:], vl_sb[:])
    n = _prune_waits(nc)
    return nc


def _prune_waits(nc):
    """Drop semaphore waits that are transitively implied by another wait on
    the same instruction.

    The walrus codegen used here allows at most ONE sync wait per
    instruction.  Tile's sem assignment is not transitively minimal: e.g. a
    consumer waits on both a DMA completion and on a PE tick even though the
    DMA itself already waited on that PE tick.  Per-proc completion is
    in-order (engine FIFOs, per-queue DMA), so "sem S reached v" implies all
    waits of every instruction on S's proc with cumulative tick <= v held.
    We compute that closure and greedily delete implied waits.
    """
    insts = [ins for bb in nc.main_func.blocks for ins in bb.instructions]
    # proc name -> ordered [(cumtick, instr)] and instr -> its waits
    proc_events = {}
    waits_of = {}
    for ins in insts:
        si = getattr(ins, "sync_info", None)
        if si is None:
            continue
        ow = list(si.on_wait or [])
        waits_of[id(ins)] = [(w.ant_name, w.wait_value) for w in ow]
        for u in (si.on_update or []):
            if getattr(u, "update_mode", None) not in ("sem-inc", "sem-add-imm"):
                continue
            lst = proc_events.setdefault(u.ant_name, [])
            prev = lst[-1][0] if lst else 0
            lst.append((prev + (u.update_value or 1), ins))

    # holds[(sem, tick_idx)] -> {sem: max_threshold} computed lazily with
    # memoization over prefix positions; iterate to fixpoint.
    import bisect

    def prefix_index(sem, v):
        lst = proc_events.get(sem)
        if not lst:
            return None
        ticks = [t for t, _ in lst]
        i = bisect.bisect_left(ticks, v)
        return i if i < len(lst) else None

    memo = {}

    def holds(sem, v, depth=0):
        """Thresholds guaranteed held once sem >= v."""
        if depth > 6:
            return {}
        i = prefix_index(sem, v)
        if i is None:
            return {}
        key = (sem, i)
        if key in memo:
            return memo[key]
        memo[key] = {}      # cut cycles conservatively
        out = {}
        # Pool (gpsimd) has multiple cores; don't assume in-order there.
        inorder = not sem.startswith("Pool")
        rng = range(i + 1) if inorder else (i,)
        for j in rng:
            _, ins = proc_events[sem][j]
            for (s2, v2) in waits_of.get(id(ins), []):
                if out.get(s2, 0) < v2:
                    out[s2] = v2
                sub = holds(s2, v2, depth + 1)
                for s3, v3 in sub.items():
                    if out.get(s3, 0) < v3:
                        out[s3] = v3
        memo[key] = out
        return out

    # cumulative tick of each instruction on its own update proc
    own_tick = {}
    for sem, lst in proc_events.items():
        for tick, ins in lst:
            own_tick[(id(ins), sem)] = tick

    pruned = 0
    for ins in insts:
        si = getattr(ins, "sync_info", None)
        if si is None or not si.on_wait or len(si.on_wait) < 2:
            continue
        ow = list(si.on_wait)
        kept = list(ow)
        for w in ow:
            if len(kept) == 1:
                break
            # same-queue FIFO: waiting on earlier completions of the very
            # queue this instruction executes on is vacuous (per-queue
            # serial execution); addresses here are disjoint anyway.
            mine = own_tick.get((id(ins), w.ant_name))
            if mine is not None and w.wait_value <= mine - 1:
                kept.remove(w)
                pruned += 1
                continue
            others = [o for o in kept if o is not w]
            for o in others:
                h = holds(o.ant_name, o.wait_value)
                if h.get(w.ant_name, 0) >= w.wait_value:
                    kept.remove(w)
                    pruned += 1
                    break
        si.on_wait = kept
    return pruned


def _host_prep(x, conv_w, centroids):
    from concourse import mybir
    bf16np = mybir.dt.np(mybir.dt.bfloat16)

    x = np.ascontiguousarray(x, dtype=np.float32)
    L = H * W
    norm = np.sqrt((x.astype(np.float64) ** 2).sum(0))
    norm = np.maximum(norm, 1e-12).astype(np.float32)       # [H,W]
    inv_norm = (1.0 / norm).astype(np.float32)
    ii = np.arange(H, dtype=np.float32)
    mi = np.minimum(H - 1 - ii, ii)
    m = np.minimum(mi[:, None], mi[None, :]).astype(np.float32)
    m2 = m * m
    minv = (m2 * m2) * inv_norm                              # [H,W]

    xpad = np.zeros((C, H + 2, W), np.float32)
    xpad[:, 1:H + 1, :] = x
    # transposed layout with norm column, padded rows
    xtn = np.zeros(((H + 2) * W, C + 1), np.float32)
    xtn[W:(H + 1) * W, 0:C] = x.reshape(C, L).T
    xtn[W:(H + 1) * W, C] = norm.reshape(L)
    invn_pad = np.zeros((H + 2) * W, np.float32)
    invn_pad[W:(H + 1) * W] = inv_norm.reshape(L)
    minv_pad = np.zeros((H + 2) * W, np.float32)
    minv_pad[W:(H + 1) * W] = minv.reshape(L)

    cwt = np.ascontiguousarray(conv_w.T, dtype=np.float32)   # [C,K]
    cent = np.ascontiguousarray(centroids, dtype=np.float32)
    identb = np.eye(128, dtype=np.float32).astype(bf16np)
    identf = np.eye(128, dtype=np.float32)
    ones = np.ones((128, 128), np.float32)

    in_maps = []
    for core in range(M):
        r0 = core * RPC
        sl = slice(r0 * W, (r0 + RPC + 2) * W)               # slab in padded coords
        sc0c = minv_pad[sl].copy()
        sc0c[0:W] = 0.0                                      # halo rows contribute 0
        sc0c[(RPC + 1) * W:] = 0.0
        in_maps.append({
            "xcl": np.ascontiguousarray(
                xpad[:, r0:r0 + RPC + 2, :].reshape(C, Ls)),
            "xlcn": np.ascontiguousarray(xtn[sl]),
            "cwt": cwt,
            "cent": cent,
            "sc0": np.ascontiguousarray(sc0c.reshape(NT, 128).T),
            "invn": np.ascontiguousarray(invn_pad[sl].reshape(NT, 128).T.copy()),
            "identb": identb,
            "identf": identf,
            "ones": ones,
        })
    return in_maps


def _ensure_ntff_hook():
    """Install the axon NTFF profile hook if the image's antenv lacks it."""
    import types
    try:
        from antenv.axon_hooks import get_axon_ntff_profile_hook  # noqa: F401
        return
    except ImportError:
        pass
    if "/root/.axon_site" not in sys.path:
        sys.path.insert(0, "/root/.axon_site")
    from trn_agent_boot.trn_boot import _ntff_profile_via_ctypes
    hook = _ntff_profile_via_ctypes("/opt/axon/libaxon_pjrt.so")
    mod = types.ModuleType("antenv.axon_hooks")
    mod.get_axon_ntff_profile_hook = lambda: hook
    mod.set_axon_ntff_profile_hook = lambda h: None
    import antenv
    antenv.axon_hooks = mod
    sys.modules["antenv.axon_hooks"] = mod


def _install_neff_cache():
    """Cache compiled NEFFs across processes, keyed by BIR content hash."""
    import hashlib
    import shutil
    import concourse.bass2jax as b2j

    orig = b2j.compile_bir_kernel
    if getattr(orig, "_neff_cached", False):
        return

    def cached(bir_json, tmpdir, neff_name="file.neff"):
        h = hashlib.sha256(
            bir_json if isinstance(bir_json, bytes) else bir_json.encode()
        ).hexdigest()[:24]
        cdir = "/tmp/neff_cache"
        os.makedirs(cdir, exist_ok=True)
        cpath = os.path.join(cdir, h + ".neff")
        if os.path.exists(cpath):
            dst = os.path.join(tmpdir, neff_name)
            os.makedirs(tmpdir, exist_ok=True)
            shutil.copy(cpath, dst)
            return dst
        out = orig(bir_json, tmpdir, neff_name=neff_name)
        shutil.copy(out, cpath)
        return out

    cached._neff_cached = True
    b2j.compile_bir_kernel = cached


def kernel(x, conv_w, centroids):
    import concourse.bass_utils as bu
    from concourse.bass_utils import run_bass_kernel_spmd
    _install_neff_cache()
    if TRACE:
        _ensure_ntff_hook()
        bu.upload_artifacts = lambda tmpdir: "local://" + tmpdir

    if "nc" not in _CACHE:
        _CACHE["nc"] = _build_nc()
    nc = _CACHE["nc"]
    in_maps = _host_prep(np.asarray(x), np.asarray(conv_w), np.asarray(centroids))
    res = run_bass_kernel_spmd(nc, in_maps, list(range(M)), trace=TRACE)
    _CACHE["last"] = res
    red = np.zeros((K, C + 1), np.float32)
    for r in res.results:
        red += np.asarray(r["y"], dtype=np.float32)
    vlad = red[:, :C] - red[:, C:C + 1] * np.asarray(centroids, np.float32)
    vlad /= np.maximum(np.sqrt((vlad ** 2).sum(1))[:, None], 1e-12)
    v = vlad.reshape(1, K * C)
    v /= np.maximum(np.sqrt((v ** 2).sum()), 1e-12)
    return v.astype(np.float32)



# revision 43
# speedup vs baseline: 2.5560x; 2.5560x over previous
"""NetVLAD (vq_codebook) Trainium2 Bass kernel, 8-way spatially sharded. v2.

Math (validated in numpy sim to rel ~2.5e-3 vs reference):
  xn = x / ||x||_C per location (host); logits = conv_w @ xn; soft = softmax_K
  fold(unfold(soft) * top2keep) == soft * cnt, cnt = 3x3 box-sum of the
  per-cluster top-2 indicator. vlad = sa2 @ xn.T - rowsum(sa2) * centroids,
  then intra + global L2 norm (host).

v2 vs baseline: all-bf16 datapath (4x faster PE matmuls, half DMA),
host-prenormalized x (no inv-norm scale on device), col-tiled K=64 matmuls
(two concurrent 64-col groups in the 128x128 PE array), Mstack transposes
folding the col-group partial-sum, 128-partition packed box-sum buffers
(two L-halves in the partition dim), paired keep/cnt transposes, VLAD in
two col groups with host-side final sum.

Sharding: H=192 rows split 8 ways (24 rows/core + 1 halo row each side).
[128, C+1] partial VLAD sums summed on host across col groups and cores.
"""
import os
import sys

sys.path.insert(0, "/opt/trn_rl_repo")
os.environ.setdefault("MYCRO_LOCAL_CACHE", "1")

import numpy as np

C, H, W, K = 512, 192, 192, 64
M = 8                      # cores
RPC = H // M               # 24 rows per core
Ls = (RPC + 2) * W         # 4992 slab locations (incl. 1 halo row each side)
NT = Ls // 128             # 39 l-tiles
CT = C // 128              # 4 c-tiles
G1 = 257                   # kb guard (odd -> v-pass offsets even for DVE 2x)
PKW = 22 * 128             # 2816 packed box-sum columns (22 tiles)
KBW = G1 + PKW + G1        # 3330 keep-buffer width
POF = 17                   # B row-group holds tiles 17..38 (l offset 2176)
NPAIR = 22                 # keep-T pair transposes (t, t+17), t=0..21
XW = 8                     # xnt DMA batching (tiles per DMA wave)

# cnt-T schedule: pairs (j, j+17) j=3..19 (A rows->tile j, B rows->tile j+17),
# then singles: tiles 0..2 from A rows, tiles 37..38 from B rows.
CNT_PAIRS = list(range(3, 20))
CNT_SINGLE_A = [0, 1, 2]
CNT_SINGLE_B = [37, 38]
# VLAD slot order = w2 availability order; host permutes xnT rows to match.
SLOT_TILES = []
for _j in CNT_PAIRS:
    SLOT_TILES += [_j, _j + POF]
SLOT_TILES += CNT_SINGLE_A + CNT_SINGLE_B        # 39 slots

TRACE = False              # set by test.py for profiling runs
_CACHE = {}


def _build_nc():
    import concourse.bass as bass
    import concourse.tile as tile
    from concourse import mybir

    f32 = mybir.dt.float32
    bf16 = mybir.dt.bfloat16
    AF = mybir.ActivationFunctionType
    OP = mybir.AluOpType
    AX = mybir.AxisListType

    fp8 = mybir.dt.float8e4
    nc = bass.Bass()
    xnb = nc.dram_tensor("xnb", [C, Ls], fp8, kind="ExternalInput")
    xnt = nc.dram_tensor("xnt", [Ls, C], fp8, kind="ExternalInput")
    cwt = nc.dram_tensor("cwt", [C, K], fp8, kind="ExternalInput")
    msk = nc.dram_tensor("msk", [128, NT], f32, kind="ExternalInput")
    identb = nc.dram_tensor("identb", [128, 128], bf16, kind="ExternalInput")
    mstack = nc.dram_tensor("mstack", [128, K], bf16, kind="ExternalInput")
    onesb = nc.dram_tensor("onesb", [128, 128], bf16, kind="ExternalInput")
    ones8 = nc.dram_tensor("ones8", [128, 8], fp8, kind="ExternalInput")
    y = nc.dram_tensor("y", [128, C + 1], f32, kind="ExternalOutput")

    with tile.TileContext(nc) as tc:
        with tc.tile_pool(name="big", bufs=1) as big:
            # persistent SBUF tensors
            xnb_sb = big.tile([128, CT * Ls], fp8, tag="xnb")
            xnt_sb = big.tile([128, NT * C], fp8, tag="xnt")
            logklb = big.tile([128, Ls], bf16, tag="logklb")
            expb = big.tile([128, NT * K], bf16, tag="expb")
            tmpb = big.tile([128, NT * K], bf16, tag="tmpb")
            keep2 = big.tile([128, NPAIR * 128], bf16, tag="keep2")
            kb = big.tile([128, KBW], bf16, tag="kb")
            kb2 = big.tile([128, KBW], bf16, tag="kb2")
            h3s = big.tile([128, KBW], bf16, tag="h3s")
            cnt2 = big.tile([128, PKW], bf16, tag="cnt2")
            cnt_lk = big.tile([128, NPAIR * 128 + 5 * K], bf16, tag="cntlk")
            w2 = big.tile([128, NT * K], fp8, tag="w2")
            trs = big.tile([128, NT * K], bf16, tag="trs")
            trs2 = big.tile([128, NT * 32], bf16, tag="trs2")
            cwt_sb = big.tile([128, CT * K], fp8, tag="cwt")
            id_sb = big.tile([128, 128], bf16, tag="ident")
            mst_sb = big.tile([128, K], bf16, tag="mstack")
            ones_sb = big.tile([128, 128], bf16, tag="ones")
            ones8_sb = big.tile([128, 8], fp8, tag="ones8")
            msk_sb = big.tile([128, NT], f32, tag="msk")
            sume = big.tile([128, NT], f32, tag="sume")
            m1b = big.tile([128, NT], f32, tag="m1b")
            m2b = big.tile([128, NT], f32, tag="m2b")
            isum = big.tile([128, NT], f32, tag="isum")
            scc = big.tile([128, NT], f32, tag="scc")
            vl_sb = big.tile([128, C + 1], f32, tag="vl")
            scr = big.tile([128, 4], f32, tag="scr")

            # small input DMAs
            nc.sync.dma_start(id_sb[:], identb[:])
            nc.sync.dma_start(mst_sb[:], mstack[:])
            nc.sync.dma_start(ones_sb[:], onesb[:])
            nc.sync.dma_start(ones8_sb[:], ones8[:])
            nc.sync.dma_start(msk_sb[:], msk[:])
            nc.sync.dma_start(
                cwt_sb[:].rearrange("p (t k) -> p t k", k=K),
                cwt[:].rearrange("(t p) k -> p t k", p=128),
            )
            # zero the keep-buffer guards (both row groups)
            nc.vector.memset(kb[:, 0:G1], 0.0)
            nc.vector.memset(kb[:, G1 + PKW:KBW], 0.0)
            # touch ops absorbing small-DMA completions so downstream compute
            # carries at most one sync wait each
            nc.scalar.copy(scr[:, 0:1], msk_sb[:, 0:1])
            nc.vector.tensor_copy(scr[:, 1:2], msk_sb[:, 1:2])

            with tc.tile_pool(name="pp", bufs=1, space="PSUM") as pp:
                pv0 = pp.tile([128, C], f32, tag="pv0", bufs=1)
                pv1 = pp.tile([128, 8], f32, tag="pv1", bufs=1)
                # warm-up burst: absorbs small-DMA waits on the PE stream and
                # spins the PE HAM up to 2.4 GHz before phase 1
                dummy = pp.tile([128, 512], f32, tag="pm", bufs=2)
                nc.tensor.matmul(dummy[0:64, 0:64], lhsT=cwt_sb[:, 0:64],
                                 rhs=cwt_sb[:, 0:64], start=True, stop=True)
                nc.tensor.matmul(dummy[0:64, 0:64], lhsT=id_sb[:, 0:64],
                                 rhs=mst_sb[:, 0:64], start=True, stop=True)
                for _ in range(10):
                    dummy = pp.tile([128, 512], f32, tag="pm", bufs=2)
                    nc.tensor.matmul(dummy[0:64, 0:64], lhsT=ones_sb[:, 0:64],
                                     rhs=ones_sb[:, 0:64], start=True, stop=True)

                # ---- phase 1: logits [K, L] col-tiled (ct0/ct2 -> psum rows
                # 0:64, ct1/ct3 -> rows 64:128), streamed over xnb chunks
                xc3 = xnb[:].rearrange("(ct p) l -> p ct l", p=128)
                xs3 = xnb_sb[:].rearrange("p (ct l) -> p ct l", l=Ls)
                DCH = 6
                csz = Ls // DCH          # 832 cols per chunk, all c-tiles
                for j in range(DCH):
                    nc.sync.dma_start(
                        xs3[:, :, j * csz:(j + 1) * csz],
                        xc3[:, :, j * csz:(j + 1) * csz],
                    )
                nblk = (Ls + 511) // 512
                touched = set()
                for b in range(nblk):
                    w = min(512, Ls - b * 512)
                    for j in range((b * 512) // csz,
                                   (b * 512 + w - 1) // csz + 1):
                        if j not in touched:
                            touched.add(j)
                            dj = pp.tile([128, 512], f32, tag="pm", bufs=2)
                            nc.tensor.matmul(
                                dj[0:64, 0:64],
                                lhsT=xnb_sb[:, j * csz:j * csz + 64],
                                rhs=xnb_sb[:, j * csz:j * csz + 64],
                                start=True, stop=True)
                    plog = pp.tile([128, 512], f32, tag="pm", bufs=2)
                    for ct in range(CT):
                        nc.tensor.matmul(
                            plog[(ct % 2) * 64:(ct % 2) * 64 + 64, 0:w],
                            lhsT=cwt_sb[:, ct * K:(ct + 1) * K],
                            rhs=xnb_sb[:, ct * Ls + b * 512:
                                       ct * Ls + b * 512 + w],
                            start=(ct < 2),
                            stop=(ct >= 2),
                            tile_position=(0, (ct % 2) * 64),
                            skip_group_check=True,
                        )
                    nc.scalar.copy(logklb[:, b * 512:b * 512 + w],
                                   plog[:, 0:w])

                # ---- phase 2: transpose logits tiles with Mstack (sums the
                # two col-group partials) then exp; 8 tiles batched per PSUM
                # bank so one activation covers 8 tiles
                t = 0
                while t < NT:
                    n = min(8, NT - t)
                    pt = pp.tile([128, 8 * K], f32, tag="pt", bufs=4)
                    for i in range(n):
                        # regular matmul: out = logklb_tile.T @ Mstack sums
                        # the two col-group partials while transposing
                        nc.tensor.matmul(
                            pt[:, i * K:(i + 1) * K],
                            lhsT=logklb[:, (t + i) * 128:(t + i + 1) * 128],
                            rhs=mst_sb[:], start=True, stop=True,
                            skip_group_check=True)
                    nc.scalar.activation(
                        expb[:, t * K:(t + n) * K], pt[:, 0:n * K], AF.Exp)
                    t += n

                # ---- phase 3: batched top-2 + softmax denominators
                e3 = expb[:].rearrange("p (t k) -> p t k", k=K)
                m2bc = m2b[:][:, :, None].broadcast_to([128, NT, K])
                k4 = keep2[:].rearrange("p (t k) -> p t k", k=128)
                hiv = tmpb[:].rearrange("p (t k) -> p t k", k=K)
                lov = trs[:].rearrange("p (t k) -> p t k", k=K)
                tt2 = trs2[:].rearrange("p (t k) -> p t k", k=32)

                def ham_keep(src):
                    # small matmul reading `src`: keeps the PE HAM warm during
                    # the DVE-heavy phases and absorbs that engine's sem on
                    # the PE stream (later PE waits become prunable)
                    n = min(src.shape[-1], 64)
                    dh = pp.tile([128, 512], f32, tag="pm", bufs=2)
                    nc.tensor.matmul(dh[0:n, 0:n], lhsT=src[:, 0:n],
                                     rhs=src[:, 0:n], start=True, stop=True)

                def top2_stripe(ts, te):
                    # tournament second-max: hi = block max, lo = block
                    # second-max; merging two blocks:
                    #   hi' = max(hi_l, hi_r)
                    #   lo' = max(min(hi_l, hi_r), max(lo_l, lo_r))
                    # all stages run in DVE 2x mode (bf16, aligned halves)
                    nc.vector.tensor_tensor(
                        hiv[:, ts:te, 0:32], e3[:, ts:te, 0:32],
                        e3[:, ts:te, 32:64], op=OP.max)
                    nc.vector.tensor_tensor(
                        lov[:, ts:te, 0:32], e3[:, ts:te, 0:32],
                        e3[:, ts:te, 32:64], op=OP.min)
                    off, w = 0, 32
                    while w > 1:
                        nw = w // 2
                        noff = off + w
                        hl = hiv[:, ts:te, off:off + nw]
                        hr = hiv[:, ts:te, off + nw:off + w]
                        ll = lov[:, ts:te, off:off + nw]
                        lr = lov[:, ts:te, off + nw:off + w]
                        t1 = tt2[:, ts:te, 0:nw]
                        t2 = tt2[:, ts:te, nw:2 * nw]
                        nc.vector.tensor_tensor(t1, hl, hr, op=OP.min)
                        nc.vector.tensor_tensor(t2, ll, lr, op=OP.max)
                        if nw == 1:
                            nc.vector.tensor_tensor(
                                m2b[:, ts:te][:, :, None], t1, t2, op=OP.max)
                        else:
                            nc.vector.tensor_tensor(
                                lov[:, ts:te, noff:noff + nw], t1, t2,
                                op=OP.max)
                            nc.vector.tensor_tensor(
                                hiv[:, ts:te, noff:noff + nw], hl, hr,
                                op=OP.max)
                        off, w = noff, nw

                # two L stripes so stripe A's keep-T/kb work overlaps stripe
                # B's top-2 chain (stripe A = tiles 0..21, B = 22..38)
                for (ts, te) in ((0, NPAIR), (NPAIR, NT)):
                    top2_stripe(ts, te)
                    ham_keep(tmpb[:, 0:32])
                    if ts == 0:
                        # keep pair-slot cols 0:64 = tiles 0..21 (stripe A)
                        # and cols 64:128 of slots 0..4 = tiles 17..21
                        nc.vector.tensor_tensor(
                            k4[:, :, 0:K], e3[:, 0:NPAIR], m2bc[:, 0:NPAIR],
                            op=OP.is_ge)
                        nc.vector.tensor_tensor(
                            k4[:, 0:5, K:128], e3[:, POF:POF + 5],
                            m2bc[:, POF:POF + 5], op=OP.is_ge)
                    else:
                        # cols 64:128 of slots 5..21 = tiles 22..38 (stripe B)
                        nc.vector.tensor_tensor(
                            k4[:, 5:NPAIR, K:128], e3[:, POF + 5:NT],
                            m2bc[:, POF + 5:NT], op=OP.is_ge)

                # ---- phase 4: keep-T pair transposes into the packed
                # guarded buffer: kb rows 0:64 = keep[K, l] for l tiles 0..21,
                # rows 64:128 = tiles 17..38, same columns
                for t in range(NPAIR):
                    pk = pp.tile([128, 128], bf16, tag="pt", bufs=4)
                    nc.tensor.transpose(
                        pk[:], keep2[:, t * 128:(t + 1) * 128], id_sb[:])
                    nc.scalar.copy(kb[:, G1 + t * 128:G1 + (t + 1) * 128],
                                   pk[:])
                    if t % 6 == 5:   # keep the PE HAM warm through this phase
                        dh = pp.tile([128, 512], f32, tag="pm", bufs=2)
                        nc.tensor.matmul(
                            dh[0:64, 0:64], lhsT=ones_sb[:, 0:64],
                            rhs=ones_sb[:, 0:64], start=True, stop=True)

                # softmax denominators off the critical chain: they run on
                # DVE while the PE does keep-T transposes
                nc.vector.tensor_reduce(sume[:], e3, axis=AX.X, op=OP.add)
                nc.vector.reciprocal(isum[:], sume[:])
                nc.vector.tensor_mul(scc[:], msk_sb[:], isum[:])

                # ---- phase 5: separable 3x3 box-sum on the packed buffer,
                # in two column chunks so chunk 0 starts while late keep-T
                # copies are still landing. kb2 = kb shifted by one
                # (4x-mode copy) so all h-pass taps are 4B-aligned (DVE 2x).
                # h3s[j] = kb[j] + kb[j+1] + kb[j+2]  (h[j+1], shifted)
                # cnt[c] = h[G1+c-192] + h[G1+c] + h[G1+c+192], h = h3s[j-1]
                HSPLIT = 1856
                VSPLIT = 1408
                for ci in range(2):
                    h0, h1 = (0, HSPLIT) if ci == 0 else (HSPLIT, KBW - 2)
                    c0, c1 = (0, VSPLIT) if ci == 0 else (VSPLIT, PKW)
                    nc.vector.tensor_copy(kb2[:, h0:h1], kb[:, h0 + 1:h1 + 1])
                    nc.vector.tensor_add(
                        h3s[:, h0:h1], kb[:, h0:h1], kb[:, h0 + 2:h1 + 2])
                    nc.vector.tensor_add(
                        h3s[:, h0:h1], h3s[:, h0:h1], kb2[:, h0:h1])
                    nc.vector.tensor_add(
                        cnt2[:, c0:c1], h3s[:, G1 - 193 + c0:G1 - 193 + c1],
                        h3s[:, G1 + 191 + c0:G1 + 191 + c1])
                    nc.vector.tensor_add(
                        cnt2[:, c0:c1], cnt2[:, c0:c1],
                        h3s[:, G1 - 1 + c0:G1 - 1 + c1])
                    ham_keep(cnt2[:, c0:c0 + 64])

                # ---- phase 6: cnt-T back to [l, K], fuse w2 = cnt*scc*exp,
                # and immediately accumulate VLAD for each finished tile.
                # xnT arrives permuted in VLAD slot order (5 waves).
                x3 = xnt[:].rearrange("(a p) c -> p a c", p=128)
                xt3 = xnt_sb[:].rearrange("p (a c) -> p a c", c=C)
                nwav = (NT + XW - 1) // XW
                for wv in range(nwav):
                    n = min(XW, NT - wv * XW)
                    nc.sync.dma_start(
                        xt3[:, wv * XW:wv * XW + n, :],
                        x3[:, wv * XW:wv * XW + n, :],
                    )

                slot = 0
                started = [False, False]   # col group A (tiles<20), B

                def vlad_mm(tl, last):
                    nonlocal slot
                    grp = 0 if tl < 20 else 1
                    rows = slice(grp * 64, grp * 64 + 64)
                    lt = w2[:, tl * K:(tl + 1) * K]
                    if slot % XW == 0:     # absorb this wave's DMA wait
                        dw = pp.tile([128, 512], f32, tag="pm", bufs=2)
                        nc.tensor.matmul(
                            dw[0:64, 0:64],
                            lhsT=xnt_sb[:, slot * C:slot * C + 64],
                            rhs=xnt_sb[:, slot * C:slot * C + 64],
                            start=True, stop=True)
                    nc.tensor.matmul(
                        pv0[rows, :], lhsT=lt,
                        rhs=xnt_sb[:, slot * C:(slot + 1) * C],
                        start=not started[grp], stop=last,
                        tile_position=(0, grp * 64),
                        skip_group_check=True)
                    nc.tensor.matmul(
                        pv1[rows, 0:1], lhsT=lt, rhs=ones8_sb[:, 0:1],
                        start=not started[grp], stop=last,
                        tile_position=(0, grp * 64),
                        skip_group_check=True)
                    started[grp] = True
                    slot += 1

                def w2_fuse(tl, src, eng=None):
                    # alternate DVE / gpsimd so neither engine gates the tail
                    (eng or nc.vector).scalar_tensor_tensor(
                        w2[:, tl * K:(tl + 1) * K], src,
                        scc[:, tl:tl + 1], expb[:, tl * K:(tl + 1) * K],
                        op0=OP.mult, op1=OP.mult)

                # work items: 17 pair transposes then 5 singles; transposes
                # are emitted 3 ahead of their consumers (pt bufs=4) so the
                # PE never stalls on the act/DVE pipeline behind it
                def emit_T(i):
                    pc = pp.tile([128, 128], bf16, tag="pt", bufs=4)
                    if i < len(CNT_PAIRS):
                        j = CNT_PAIRS[i]
                        nc.tensor.transpose(
                            pc[:], cnt2[:, j * 128:(j + 1) * 128], id_sb[:])
                    else:
                        t = (CNT_SINGLE_A + CNT_SINGLE_B)[i - len(CNT_PAIRS)]
                        if t < 20:
                            nc.tensor.transpose(
                                pc[:, 0:K], cnt2[0:64, t * 128:(t + 1) * 128],
                                id_sb[0:64, 0:K])
                        else:
                            nc.tensor.transpose(
                                pc[:, 0:K],
                                cnt2[64:128,
                                     (t - POF) * 128:(t - POF + 1) * 128],
                                id_sb[64:128, 64:64 + K])
                    return pc

                def consume(i, pc):
                    if i < len(CNT_PAIRS):
                        j = CNT_PAIRS[i]
                        cl = cnt_lk[:, i * 128:(i + 1) * 128]
                        nc.scalar.copy(cl, pc[:])
                        tA, tB = j, j + POF
                        w2_fuse(tA, cl[:, 0:K])
                        w2_fuse(tB, cl[:, K:128])
                        vlad_mm(tA, last=False)
                        vlad_mm(tB, last=False)
                    else:
                        t = (CNT_SINGLE_A + CNT_SINGLE_B)[i - len(CNT_PAIRS)]
                        cl = cnt_lk[:, NPAIR * 128 + (i - len(CNT_PAIRS)) * K:
                                    NPAIR * 128 + (i - len(CNT_PAIRS) + 1) * K]
                        nc.scalar.copy(cl, pc[:, 0:K])
                        w2_fuse(t, cl)
                        vlad_mm(t, last=(t in (2, 38)))

                NW = len(CNT_PAIRS) + 5
                pend = []
                for i in range(min(3, NW)):
                    pend.append(emit_T(i))
                for i in range(NW):
                    consume(i, pend[i])
                    if i + 3 < NW:
                        pend.append(emit_T(i + 3))

                # ---- phase 7: write this core's [128, C+1] partial sums;
                # host sums col groups + cores, applies centroid subtraction
                # and the two L2 normalizations
                nc.scalar.copy(vl_sb[:, 0:C], pv0[:])
                nc.scalar.copy(vl_sb[:, C:C + 1], pv1[:, 0:1])
                nc.sync.dma_start(y[:], vl_sb[:])
    _prune_waits(nc)
    return nc


def _prune_waits(nc):
    """Drop semaphore waits that are transitively implied by another wait on
    the same instruction (walrus codegen allows one hw wait per compute
    instruction; extra waits cost separate EVENT_SEMAPHORE instructions)."""
    insts = [ins for bb in nc.main_func.blocks for ins in bb.instructions]
    proc_events = {}
    waits_of = {}
    pending = {}    # engine -> waits of non-ticking instrs (e.g. Ldweights),
    #                 folded into the next ticking instr on that engine so the
    #                 transitive closure can see them (engines run in-order)
    for ins in insts:
        si = getattr(ins, "sync_info", None)
        if si is None:
            continue
        eng = getattr(ins, "engine", None)
        ow = [(w.ant_name, w.wait_value) for w in (si.on_wait or [])]
        carried = pending.get(eng, [])
        all_waits = carried + ow
        ticked = False
        for u in (si.on_update or []):
            if getattr(u, "update_mode", None) not in ("sem-inc", "sem-add-imm"):
                continue
            ticked = True
            lst = proc_events.setdefault(u.ant_name, [])
            prev = lst[-1][0] if lst else 0
            lst.append((prev + (u.update_value or 1), ins))
        waits_of[id(ins)] = all_waits if ticked else ow
        pending[eng] = [] if ticked else all_waits

    import bisect

    def prefix_index(sem, v):
        lst = proc_events.get(sem)
        if not lst:
            return None
        ticks = [t for t, _ in lst]
        i = bisect.bisect_left(ticks, v)
        return i if i < len(lst) else None

    memo = {}

    def holds(sem, v, depth=0):
        if depth > 6:
            return {}
        i = prefix_index(sem, v)
        if i is None:
            return {}
        key = (sem, i)
        if key in memo:
            return memo[key]
        memo[key] = {}
        out = {}
        inorder = not sem.startswith("Pool")
        rng = range(i + 1) if inorder else (i,)
        for j in rng:
            _, ins = proc_events[sem][j]
            for (s2, v2) in waits_of.get(id(ins), []):
                if out.get(s2, 0) < v2:
                    out[s2] = v2
                sub = holds(s2, v2, depth + 1)
                for s3, v3 in sub.items():
                    if out.get(s3, 0) < v3:
                        out[s3] = v3
        memo[key] = out
        return out

    own_tick = {}
    for sem, lst in proc_events.items():
        for tick, ins in lst:
            own_tick[(id(ins), sem)] = tick

    pruned = 0
    for ins in insts:
        si = getattr(ins, "sync_info", None)
        if si is None or not si.on_wait or len(si.on_wait) < 2:
            continue
        ow = list(si.on_wait)
        kept = list(ow)
        tn = type(ins).__name__
        is_dma = "DMA" in tn or "Drain" in tn
        for w in ow:
            if len(kept) == 1:
                break
            # same-queue FIFO rule, DMA instructions only: waiting on earlier
            # completions of the queue this DMA executes on is vacuous
            # (per-queue serial execution). Compute engines keep such waits:
            # the race detector requires them when APs overlap.
            if is_dma:
                mine = own_tick.get((id(ins), w.ant_name))
                if mine is not None and w.wait_value <= mine - 1:
                    kept.remove(w)
                    pruned += 1
                    continue
            others = [o for o in kept if o is not w]
            for o in others:
                h = holds(o.ant_name, o.wait_value)
                if h.get(w.ant_name, 0) >= w.wait_value:
                    kept.remove(w)
                    pruned += 1
                    break
        si.on_wait = kept
    return pruned


def _host_prep(x, conv_w, centroids):
    from concourse import mybir
    bf16np = mybir.dt.np(mybir.dt.bfloat16)
    fp8np = mybir.dt.np(mybir.dt.float8e4)

    x = np.ascontiguousarray(x, dtype=np.float32)
    norm = np.sqrt((x.astype(np.float64) ** 2).sum(0))
    xn = (x / np.maximum(norm, 1e-12)).astype(np.float32)    # [C,H,W]
    ii = np.arange(H, dtype=np.float64)
    mi = np.minimum(H - 1 - ii, ii)
    m = np.minimum(mi[:, None], mi[None, :])
    m4 = m ** 4
    # rescale so w2 = msk*soft*cnt fits fp8e4m3 range; the global scale
    # cancels in the intra-cluster L2 normalization on the host
    msk_full = (m4 / m4.max()).astype(np.float32)            # [H,W]

    xn_pad = np.zeros((C, H + 2, W), np.float32)
    xn_pad[:, 1:H + 1] = xn
    msk_pad = np.zeros((H + 2, W), np.float32)
    msk_pad[1:H + 1] = msk_full

    cwtb = np.ascontiguousarray(conv_w.T).astype(fp8np)      # [C,K]
    identb = np.eye(128, dtype=np.float32).astype(bf16np)
    mstack = np.concatenate([np.eye(K), np.eye(K)], 0).astype(np.float32)
    mstack = mstack.astype(bf16np)                           # [128, K]
    onesb = np.ones((128, 128), np.float32).astype(bf16np)
    slot = np.array(SLOT_TILES)

    in_maps = []
    for core in range(M):
        r0 = core * RPC
        slab = np.ascontiguousarray(
            xn_pad[:, r0:r0 + RPC + 2, :].reshape(C, Ls))
        mskc = msk_pad[r0:r0 + RPC + 2].reshape(Ls).copy()
        mskc[0:W] = 0.0
        mskc[(RPC + 1) * W:] = 0.0                           # halo rows -> 0
        xnT = np.ascontiguousarray(slab.T).astype(fp8np)     # [Ls, C]
        # permute l-tiles into VLAD slot order
        xnT_perm = np.ascontiguousarray(
            xnT.reshape(NT, 128, C)[slot].reshape(Ls, C))
        in_maps.append({
            "xnb": slab.astype(fp8np),
            "xnt": xnT_perm,
            "cwt": cwtb,
            "msk": np.ascontiguousarray(mskc.reshape(NT, 128).T),
            "identb": identb,
            "mstack": mstack,
            "onesb": onesb,
            "ones8": np.ones((128, 8), np.float32).astype(fp8np),
        })
    return in_maps


def _ensure_ntff_hook():
    """Install the axon NTFF profile hook if the image's antenv lacks it."""
    import types
    try:
        from antenv.axon_hooks import get_axon_ntff_profile_hook  # noqa: F401
        return
    except ImportError:
        pass
    if "/root/.axon_site" not in sys.path:
        sys.path.insert(0, "/root/.axon_site")
    from trn_agent_boot.trn_boot import _ntff_profile_via_ctypes
    hook = _ntff_profile_via_ctypes("/opt/axon/libaxon_pjrt.so")
    mod = types.ModuleType("antenv.axon_hooks")
    mod.get_axon_ntff_profile_hook = lambda: hook
    mod.set_axon_ntff_profile_hook = lambda h: None
    import antenv
    antenv.axon_hooks = mod
    sys.modules["antenv.axon_hooks"] = mod


def _install_neff_cache():
    """Cache compiled NEFFs across processes, keyed by BIR content hash."""
    import hashlib
    import shutil
    import concourse.bass2jax as b2j

    orig = b2j.compile_bir_kernel
    if getattr(orig, "_neff_cached", False):
        return

    def cached(bir_json, tmpdir, neff_name="file.neff"):
        h = hashlib.sha256(
            bir_json if isinstance(bir_json, bytes) else bir_json.encode()
        ).hexdigest()[:24]
        cdir = "/tmp/neff_cache"
        os.makedirs(cdir, exist_ok=True)
        cpath = os.path.join(cdir, h + ".neff")
        if os.path.exists(cpath):
            dst = os.path.join(tmpdir, neff_name)
            os.makedirs(tmpdir, exist_ok=True)
            shutil.copy(cpath, dst)
            return dst
        out = orig(bir_json, tmpdir, neff_name=neff_name)
        shutil.copy(out, cpath)
        return out

    cached._neff_cached = True
    b2j.compile_bir_kernel = cached


def kernel(x, conv_w, centroids):
    import concourse.bass_utils as bu
    from concourse.bass_utils import run_bass_kernel_spmd
    _install_neff_cache()
    if TRACE:
        _ensure_ntff_hook()
        bu.upload_artifacts = lambda tmpdir: "local://" + tmpdir

    if "nc" not in _CACHE:
        _CACHE["nc"] = _build_nc()
    nc = _CACHE["nc"]
    in_maps = _host_prep(np.asarray(x), np.asarray(conv_w), np.asarray(centroids))
    res = run_bass_kernel_spmd(nc, in_maps, list(range(M)), trace=TRACE)
    _CACHE["last"] = res
    red = np.zeros((128, C + 1), np.float64)
    for r in res.results:
        red += np.asarray(r["y"], dtype=np.float64)
    redk = red[0:64] + red[64:128]                           # [K, C+1]
    vlad = redk[:, :C] - redk[:, C:C + 1] * np.asarray(centroids, np.float64)
    vlad /= np.maximum(np.sqrt((vlad ** 2).sum(1))[:, None], 1e-12)
    v = vlad.reshape(1, K * C)
    v /= np.maximum(np.sqrt((v ** 2).sum()), 1e-12)
    return v.astype(np.float32)


# revision 49
# speedup vs baseline: 2.7632x; 1.0811x over previous
"""NetVLAD (vq_codebook) Trainium2 Bass kernel, 8-way spatially sharded. v2.

Math (validated in numpy sim to rel ~2.5e-3 vs reference):
  xn = x / ||x||_C per location (host); logits = conv_w @ xn; soft = softmax_K
  fold(unfold(soft) * top2keep) == soft * cnt, cnt = 3x3 box-sum of the
  per-cluster top-2 indicator. vlad = sa2 @ xn.T - rowsum(sa2) * centroids,
  then intra + global L2 norm (host).

v2 vs baseline: all-bf16 datapath (4x faster PE matmuls, half DMA),
host-prenormalized x (no inv-norm scale on device), col-tiled K=64 matmuls
(two concurrent 64-col groups in the 128x128 PE array), Mstack transposes
folding the col-group partial-sum, 128-partition packed box-sum buffers
(two L-halves in the partition dim), paired keep/cnt transposes, VLAD in
two col groups with host-side final sum.

Sharding: H=192 rows split 8 ways (24 rows/core + 1 halo row each side).
[128, C+1] partial VLAD sums summed on host across col groups and cores.
"""
import os
import sys

sys.path.insert(0, "/opt/trn_rl_repo")
os.environ.setdefault("MYCRO_LOCAL_CACHE", "1")

import numpy as np

C, H, W, K = 512, 192, 192, 64
M = 8                      # cores
RPC = H // M               # 24 rows per core
Ls = (RPC + 2) * W         # 4992 slab locations (incl. 1 halo row each side)
NT = Ls // 128             # 39 l-tiles
CT = C // 128              # 4 c-tiles
G1 = 257                   # kb guard (odd -> v-pass offsets even for DVE 2x)
PKW = 22 * 128             # 2816 packed box-sum columns (22 tiles)
KBW = G1 + PKW + G1        # 3330 keep-buffer width
POF = 17                   # B row-group holds tiles 17..38 (l offset 2176)
NPAIR = 22                 # keep-T pair transposes (t, t+17), t=0..21
XW = 8                     # xnt DMA batching (tiles per DMA wave)

# cnt-T schedule: pairs (j, j+17) j=3..19 (A rows->tile j, B rows->tile j+17),
# then singles: tiles 0..2 from A rows, tiles 37..38 from B rows.
CNT_PAIRS = list(range(3, 20))
CNT_SINGLE_A = [0, 1, 2]
CNT_SINGLE_B = [37, 38]
# VLAD slot order = w2 availability order; host permutes xnT rows to match.
SLOT_TILES = []
for _j in CNT_PAIRS:
    SLOT_TILES += [_j, _j + POF]
SLOT_TILES += CNT_SINGLE_A + CNT_SINGLE_B        # 39 slots

TRACE = False              # set by test.py for profiling runs
_CACHE = {}


def _build_nc():
    import concourse.bass as bass
    import concourse.tile as tile
    from concourse import mybir

    f32 = mybir.dt.float32
    bf16 = mybir.dt.bfloat16
    AF = mybir.ActivationFunctionType
    OP = mybir.AluOpType
    AX = mybir.AxisListType

    fp8 = mybir.dt.float8e4
    nc = bass.Bass()
    xnb = nc.dram_tensor("xnb", [C, Ls], fp8, kind="ExternalInput")
    xnt = nc.dram_tensor("xnt", [Ls, C], fp8, kind="ExternalInput")
    # small inputs packed into two tensors (fewer serialized DMA triggers):
    # smallb = identb(128) | mstack(64) | msk-as-bf16(NT) cols, bf16
    # small8 = cwt(4*64) | ones8(8) cols, fp8
    smallb = nc.dram_tensor("smallb", [128, 128 + K + NT], bf16,
                            kind="ExternalInput")
    small8 = nc.dram_tensor("small8", [128, CT * K + 8], fp8,
                            kind="ExternalInput")
    y = nc.dram_tensor("y", [128, C + 1], f32, kind="ExternalOutput")

    with tile.TileContext(nc) as tc:
        with tc.tile_pool(name="big", bufs=1) as big:
            # persistent SBUF tensors
            xnb_sb = big.tile([128, CT * Ls], fp8, tag="xnb")
            xnt_sb = big.tile([128, NT * C], fp8, tag="xnt")
            logklb = big.tile([128, Ls], bf16, tag="logklb")
            expb = big.tile([128, NT * K], bf16, tag="expb")
            tmpb = big.tile([128, NT * K], bf16, tag="tmpb")
            keep2 = big.tile([128, NPAIR * 128], bf16, tag="keep2")
            kb = big.tile([128, KBW], bf16, tag="kb")
            kb2 = big.tile([128, KBW], bf16, tag="kb2")
            h3s = big.tile([128, KBW], bf16, tag="h3s")
            cnt2 = big.tile([128, PKW], bf16, tag="cnt2")
            cnt_lk = big.tile([128, NPAIR * 128 + 5 * K], bf16, tag="cntlk")
            w2 = big.tile([128, NT * K], fp8, tag="w2")
            trs = big.tile([128, NT * K], bf16, tag="trs")
            trs2 = big.tile([128, NT * 32], bf16, tag="trs2")
            smallb_sb = big.tile([128, 128 + K + NT], bf16, tag="smallb")
            small8_sb = big.tile([128, CT * K + 8], fp8, tag="small8")
            id_sb = smallb_sb[:, 0:128]
            mst_sb = smallb_sb[:, 128:128 + K]
            msk_sb = smallb_sb[:, 128 + K:128 + K + NT]
            cwt_sb = small8_sb[:, 0:CT * K]
            ones8_sb = small8_sb[:, CT * K:CT * K + 8]
            warm = big.tile([128, 512], bf16, tag="warm")
            sume = big.tile([128, NT], f32, tag="sume")
            m1b = big.tile([128, NT], f32, tag="m1b")
            m2b = big.tile([128, NT], f32, tag="m2b")
            isum = big.tile([128, NT], f32, tag="isum")
            scc = big.tile([128, NT], f32, tag="scc")
            vl_sb = big.tile([128, C + 1], f32, tag="vl")
            scr = big.tile([128, 4], f32, tag="scr")

            # zero the keep-buffer guards (both row groups); build the PE
            # warm-up operand without any DMA dependency
            nc.vector.memset(kb[:, 0:G1], 0.0)
            nc.vector.memset(kb[:, G1 + PKW:KBW], 0.0)
            nc.vector.memset(warm[:], 1.0)

            # DMA order: first xnb chunk first (it gates the logits), then
            # the two packed small inputs, then the remaining chunks
            xc3 = xnb[:].rearrange("(ct p) l -> p ct l", p=128)
            xs3 = xnb_sb[:].rearrange("p (ct l) -> p ct l", l=Ls)
            DCH = 6
            csz = Ls // DCH              # 832 cols per chunk, all c-tiles
            nc.sync.dma_start(xs3[:, :, 0:csz], xc3[:, :, 0:csz])
            nc.sync.dma_start(smallb_sb[:], smallb[:])
            nc.sync.dma_start(small8_sb[:], small8[:])
            for j in range(1, DCH):
                nc.sync.dma_start(
                    xs3[:, :, j * csz:(j + 1) * csz],
                    xc3[:, :, j * csz:(j + 1) * csz],
                )
            # touch ops absorbing the packed-small-DMA completions so
            # downstream compute carries at most one sync wait each
            nc.scalar.copy(scr[:, 0:1], msk_sb[:, 0:1])
            nc.vector.tensor_copy(scr[:, 1:2], msk_sb[:, 1:2])

            with tc.tile_pool(name="pp", bufs=1, space="PSUM") as pp:
                pv0 = pp.tile([128, C], f32, tag="pv0", bufs=1)
                pv1 = pp.tile([128, 8], f32, tag="pv1", bufs=1)
                # sustained warm-up burst (~4us of PE activity, no DMA deps):
                # trips the HAM to K=8/8 during the DMA-bound startup so the
                # logits matmuls run at 2.4 GHz from the first block
                for i in range(10):
                    dummy = pp.tile([128, 512], f32, tag="pm", bufs=2)
                    nc.tensor.matmul(dummy[0:64, :], lhsT=warm[:, 0:64],
                                     rhs=warm[:], start=True, stop=True)
                # absorb the packed small-input DMA sems on the PE stream
                dummy = pp.tile([128, 512], f32, tag="pm", bufs=2)
                nc.tensor.matmul(dummy[0:64, 0:64], lhsT=cwt_sb[:, 0:64],
                                 rhs=cwt_sb[:, 0:64], start=True, stop=True)
                nc.tensor.matmul(dummy[0:64, 0:64], lhsT=id_sb[:, 0:64],
                                 rhs=mst_sb[:, 0:64], start=True, stop=True)

                # ---- phase 1: logits [K, L] col-tiled (ct0/ct2 -> psum rows
                # 0:64, ct1/ct3 -> rows 64:128), streamed over xnb chunks
                nblk = (Ls + 511) // 512
                touched = set()
                for b in range(nblk):
                    w = min(512, Ls - b * 512)
                    for j in range((b * 512) // csz,
                                   (b * 512 + w - 1) // csz + 1):
                        if j not in touched:
                            touched.add(j)
                            dj = pp.tile([128, 512], f32, tag="pm", bufs=2)
                            nc.tensor.matmul(
                                dj[0:64, 0:64],
                                lhsT=xnb_sb[:, j * csz:j * csz + 64],
                                rhs=xnb_sb[:, j * csz:j * csz + 64],
                                start=True, stop=True)
                    plog = pp.tile([128, 512], f32, tag="pm", bufs=2)
                    for ct in range(CT):
                        nc.tensor.matmul(
                            plog[(ct % 2) * 64:(ct % 2) * 64 + 64, 0:w],
                            lhsT=cwt_sb[:, ct * K:(ct + 1) * K],
                            rhs=xnb_sb[:, ct * Ls + b * 512:
                                       ct * Ls + b * 512 + w],
                            start=(ct < 2),
                            stop=(ct >= 2),
                            tile_position=(0, (ct % 2) * 64),
                            skip_group_check=True,
                        )
                    nc.scalar.copy(logklb[:, b * 512:b * 512 + w],
                                   plog[:, 0:w])

                # ---- phase 2: transpose logits tiles with Mstack (sums the
                # two col-group partials) then exp; 8 tiles batched per PSUM
                # bank so one activation covers 8 tiles
                t = 0
                while t < NT:
                    n = min(8, NT - t)
                    pt = pp.tile([128, 8 * K], f32, tag="pt", bufs=4)
                    for i in range(n):
                        # regular matmul: out = logklb_tile.T @ Mstack sums
                        # the two col-group partials while transposing
                        nc.tensor.matmul(
                            pt[:, i * K:(i + 1) * K],
                            lhsT=logklb[:, (t + i) * 128:(t + i + 1) * 128],
                            rhs=mst_sb[:], start=True, stop=True,
                            skip_group_check=True)
                    nc.scalar.activation(
                        expb[:, t * K:(t + n) * K], pt[:, 0:n * K], AF.Exp)
                    t += n

                # ---- phase 3: batched top-2 + softmax denominators
                e3 = expb[:].rearrange("p (t k) -> p t k", k=K)
                m2bc = m2b[:][:, :, None].broadcast_to([128, NT, K])
                k4 = keep2[:].rearrange("p (t k) -> p t k", k=128)
                hiv = tmpb[:].rearrange("p (t k) -> p t k", k=K)
                lov = trs[:].rearrange("p (t k) -> p t k", k=K)
                tt2 = trs2[:].rearrange("p (t k) -> p t k", k=32)

                def ham_keep(src):
                    # small matmul reading `src`: keeps the PE HAM warm during
                    # the DVE-heavy phases and absorbs that engine's sem on
                    # the PE stream (later PE waits become prunable)
                    n = min(src.shape[-1], 64)
                    dh = pp.tile([128, 512], f32, tag="pm", bufs=2)
                    nc.tensor.matmul(dh[0:n, 0:n], lhsT=src[:, 0:n],
                                     rhs=src[:, 0:n], start=True, stop=True)

                def top2_stripe(ts, te):
                    # tournament second-max: hi = block max, lo = block
                    # second-max; merging two blocks:
                    #   hi' = max(hi_l, hi_r)
                    #   lo' = max(min(hi_l, hi_r), max(lo_l, lo_r))
                    # all stages run in DVE 2x mode (bf16, aligned halves)
                    nc.vector.tensor_tensor(
                        hiv[:, ts:te, 0:32], e3[:, ts:te, 0:32],
                        e3[:, ts:te, 32:64], op=OP.max)
                    nc.vector.tensor_tensor(
                        lov[:, ts:te, 0:32], e3[:, ts:te, 0:32],
                        e3[:, ts:te, 32:64], op=OP.min)
                    off, w = 0, 32
                    while w > 1:
                        nw = w // 2
                        noff = off + w
                        hl = hiv[:, ts:te, off:off + nw]
                        hr = hiv[:, ts:te, off + nw:off + w]
                        ll = lov[:, ts:te, off:off + nw]
                        lr = lov[:, ts:te, off + nw:off + w]
                        t1 = tt2[:, ts:te, 0:nw]
                        t2 = tt2[:, ts:te, nw:2 * nw]
                        nc.vector.tensor_tensor(t1, hl, hr, op=OP.min)
                        nc.vector.tensor_tensor(t2, ll, lr, op=OP.max)
                        if nw == 1:
                            nc.vector.tensor_tensor(
                                m2b[:, ts:te][:, :, None], t1, t2, op=OP.max)
                        else:
                            nc.vector.tensor_tensor(
                                lov[:, ts:te, noff:noff + nw], t1, t2,
                                op=OP.max)
                            nc.vector.tensor_tensor(
                                hiv[:, ts:te, noff:noff + nw], hl, hr,
                                op=OP.max)
                        off, w = noff, nw

                # two L stripes so stripe A's keep-T/kb work overlaps stripe
                # B's top-2 chain (stripe A = tiles 0..21, B = 22..38)
                for (ts, te) in ((0, NPAIR), (NPAIR, NT)):
                    top2_stripe(ts, te)
                    ham_keep(tmpb[:, 0:32])
                    if ts == 0:
                        # keep pair-slot cols 0:64 = tiles 0..21 (stripe A)
                        # and cols 64:128 of slots 0..4 = tiles 17..21
                        nc.vector.tensor_tensor(
                            k4[:, :, 0:K], e3[:, 0:NPAIR], m2bc[:, 0:NPAIR],
                            op=OP.is_ge)
                        nc.vector.tensor_tensor(
                            k4[:, 0:5, K:128], e3[:, POF:POF + 5],
                            m2bc[:, POF:POF + 5], op=OP.is_ge)
                    else:
                        # cols 64:128 of slots 5..21 = tiles 22..38 (stripe B)
                        nc.vector.tensor_tensor(
                            k4[:, 5:NPAIR, K:128], e3[:, POF + 5:NT],
                            m2bc[:, POF + 5:NT], op=OP.is_ge)

                # ---- phase 4: keep-T pair transposes into the packed
                # guarded buffer: kb rows 0:64 = keep[K, l] for l tiles 0..21,
                # rows 64:128 = tiles 17..38, same columns
                for t in range(NPAIR):
                    pk = pp.tile([128, 128], bf16, tag="pt", bufs=4)
                    nc.tensor.transpose(
                        pk[:], keep2[:, t * 128:(t + 1) * 128], id_sb[:])
                    nc.scalar.copy(kb[:, G1 + t * 128:G1 + (t + 1) * 128],
                                   pk[:])
                    if t % 6 == 5:   # keep the PE HAM warm through this phase
                        dh = pp.tile([128, 512], f32, tag="pm", bufs=2)
                        nc.tensor.matmul(
                            dh[0:64, 0:64], lhsT=warm[:, 0:64],
                            rhs=warm[:, 0:64], start=True, stop=True)

                # softmax denominators off the critical chain (run on DVE
                # while the PE does keep-T transposes); halving add-tree in
                # 2x mode beats the all-1x tensor_reduce
                nc.vector.tensor_add(
                    hiv[:, :, 0:32], e3[:, :, 0:32], e3[:, :, 32:64])
                off, w = 0, 32
                while w > 1:
                    nw = w // 2
                    noff = off + w
                    dst = (sume[:][:, :, None] if nw == 1
                           else hiv[:, :, noff:noff + nw])
                    nc.vector.tensor_add(
                        dst, hiv[:, :, off:off + nw],
                        hiv[:, :, off + nw:off + w])
                    off, w = noff, nw
                nc.vector.reciprocal(isum[:], sume[:])
                nc.vector.tensor_mul(scc[:], msk_sb[:], isum[:])

                # ---- phase 5: separable 3x3 box-sum on the packed buffer,
                # in two column chunks so chunk 0 starts while late keep-T
                # copies are still landing. kb2 = kb shifted by one
                # (4x-mode copy) so all h-pass taps are 4B-aligned (DVE 2x).
                # h3s[j] = kb[j] + kb[j+1] + kb[j+2]  (h[j+1], shifted)
                # cnt[c] = h[G1+c-192] + h[G1+c] + h[G1+c+192], h = h3s[j-1]
                HSPLIT = 1856
                VSPLIT = 1408
                for ci in range(2):
                    h0, h1 = (0, HSPLIT) if ci == 0 else (HSPLIT, KBW - 2)
                    c0, c1 = (0, VSPLIT) if ci == 0 else (VSPLIT, PKW)
                    nc.vector.tensor_copy(kb2[:, h0:h1], kb[:, h0 + 1:h1 + 1])
                    nc.vector.tensor_add(
                        h3s[:, h0:h1], kb[:, h0:h1], kb[:, h0 + 2:h1 + 2])
                    nc.vector.tensor_add(
                        h3s[:, h0:h1], h3s[:, h0:h1], kb2[:, h0:h1])
                    nc.vector.tensor_add(
                        cnt2[:, c0:c1], h3s[:, G1 - 193 + c0:G1 - 193 + c1],
                        h3s[:, G1 + 191 + c0:G1 + 191 + c1])
                    nc.vector.tensor_add(
                        cnt2[:, c0:c1], cnt2[:, c0:c1],
                        h3s[:, G1 - 1 + c0:G1 - 1 + c1])
                    if ci == 0:
                        # sustained re-warm burst riding the rest of the box
                        # phase so the VLAD tail starts at K=8/8; the first
                        # matmul's h3s read sequences it after the h-pass
                        dh = pp.tile([128, 512], f32, tag="pm", bufs=2)
                        nc.tensor.matmul(
                            dh[0:64, 0:64], lhsT=h3s[:, 0:64],
                            rhs=h3s[:, 0:64], start=True, stop=True)
                        for _ in range(9):
                            dh = pp.tile([128, 512], f32, tag="pm", bufs=2)
                            nc.tensor.matmul(
                                dh[0:64, :], lhsT=warm[:, 0:64], rhs=warm[:],
                                start=True, stop=True)
                    else:
                        ham_keep(cnt2[:, c0:c0 + 64])

                # ---- phase 6: cnt-T back to [l, K], fuse w2 = cnt*scc*exp,
                # and immediately accumulate VLAD for each finished tile.
                # xnT arrives permuted in VLAD slot order (5 waves).
                x3 = xnt[:].rearrange("(a p) c -> p a c", p=128)
                xt3 = xnt_sb[:].rearrange("p (a c) -> p a c", c=C)
                nwav = (NT + XW - 1) // XW
                for wv in range(nwav):
                    n = min(XW, NT - wv * XW)
                    nc.sync.dma_start(
                        xt3[:, wv * XW:wv * XW + n, :],
                        x3[:, wv * XW:wv * XW + n, :],
                    )

                slot = 0
                started = [False, False]   # col group A (tiles<20), B

                def vlad_mm(tl, last):
                    nonlocal slot
                    grp = 0 if tl < 20 else 1
                    rows = slice(grp * 64, grp * 64 + 64)
                    lt = w2[:, tl * K:(tl + 1) * K]
                    if slot % XW == 0:     # absorb this wave's DMA wait
                        dw = pp.tile([128, 512], f32, tag="pm", bufs=2)
                        nc.tensor.matmul(
                            dw[0:64, 0:64],
                            lhsT=xnt_sb[:, slot * C:slot * C + 64],
                            rhs=xnt_sb[:, slot * C:slot * C + 64],
                            start=True, stop=True)
                    nc.tensor.matmul(
                        pv0[rows, :], lhsT=lt,
                        rhs=xnt_sb[:, slot * C:(slot + 1) * C],
                        start=not started[grp], stop=last,
                        tile_position=(0, grp * 64),
                        skip_group_check=True)
                    nc.tensor.matmul(
                        pv1[rows, 0:1], lhsT=lt, rhs=ones8_sb[:, 0:1],
                        start=not started[grp], stop=last,
                        tile_position=(0, grp * 64),
                        skip_group_check=True)
                    started[grp] = True
                    slot += 1

                def w2_fuse(tl, src, eng=None):
                    # alternate DVE / gpsimd so neither engine gates the tail
                    (eng or nc.vector).scalar_tensor_tensor(
                        w2[:, tl * K:(tl + 1) * K], src,
                        scc[:, tl:tl + 1], expb[:, tl * K:(tl + 1) * K],
                        op0=OP.mult, op1=OP.mult)

                # work items: 17 pair transposes then 5 singles; transposes
                # are emitted 3 ahead of their consumers (pt bufs=4) so the
                # PE never stalls on the act/DVE pipeline behind it
                def emit_T(i):
                    pc = pp.tile([128, 128], bf16, tag="pt", bufs=4)
                    if i < len(CNT_PAIRS):
                        j = CNT_PAIRS[i]
                        nc.tensor.transpose(
                            pc[:], cnt2[:, j * 128:(j + 1) * 128], id_sb[:])
                    else:
                        t = (CNT_SINGLE_A + CNT_SINGLE_B)[i - len(CNT_PAIRS)]
                        if t < 20:
                            nc.tensor.transpose(
                                pc[:, 0:K], cnt2[0:64, t * 128:(t + 1) * 128],
                                id_sb[0:64, 0:K])
                        else:
                            nc.tensor.transpose(
                                pc[:, 0:K],
                                cnt2[64:128,
                                     (t - POF) * 128:(t - POF + 1) * 128],
                                id_sb[64:128, 64:64 + K])
                    return pc

                def consume(i, pc):
                    if i < len(CNT_PAIRS):
                        j = CNT_PAIRS[i]
                        cl = cnt_lk[:, i * 128:(i + 1) * 128]
                        nc.scalar.copy(cl, pc[:])
                        tA, tB = j, j + POF
                        w2_fuse(tA, cl[:, 0:K])
                        w2_fuse(tB, cl[:, K:128])
                        vlad_mm(tA, last=False)
                        vlad_mm(tB, last=False)
                    else:
                        t = (CNT_SINGLE_A + CNT_SINGLE_B)[i - len(CNT_PAIRS)]
                        cl = cnt_lk[:, NPAIR * 128 + (i - len(CNT_PAIRS)) * K:
                                    NPAIR * 128 + (i - len(CNT_PAIRS) + 1) * K]
                        nc.scalar.copy(cl, pc[:, 0:K])
                        w2_fuse(t, cl)
                        vlad_mm(t, last=(t in (2, 38)))

                NW = len(CNT_PAIRS) + 5
                pend = []
                for i in range(min(3, NW)):
                    pend.append(emit_T(i))
                for i in range(NW):
                    consume(i, pend[i])
                    if i + 3 < NW:
                        pend.append(emit_T(i + 3))

                # ---- phase 7: write this core's [128, C+1] partial sums;
                # host sums col groups + cores, applies centroid subtraction
                # and the two L2 normalizations
                nc.scalar.copy(vl_sb[:, 0:C], pv0[:])
                nc.scalar.copy(vl_sb[:, C:C + 1], pv1[:, 0:1])
                nc.sync.dma_start(y[:], vl_sb[:])
    _prune_waits(nc)
    return nc


def _prune_waits(nc):
    """Drop semaphore waits that are transitively implied by another wait on
    the same instruction (walrus codegen allows one hw wait per compute
    instruction; extra waits cost separate EVENT_SEMAPHORE instructions)."""
    insts = [ins for bb in nc.main_func.blocks for ins in bb.instructions]
    proc_events = {}
    waits_of = {}
    pending = {}    # engine -> waits of non-ticking instrs (e.g. Ldweights),
    #                 folded into the next ticking instr on that engine so the
    #                 transitive closure can see them (engines run in-order)
    for ins in insts:
        si = getattr(ins, "sync_info", None)
        if si is None:
            continue
        eng = getattr(ins, "engine", None)
        ow = [(w.ant_name, w.wait_value) for w in (si.on_wait or [])]
        carried = pending.get(eng, [])
        all_waits = carried + ow
        ticked = False
        for u in (si.on_update or []):
            if getattr(u, "update_mode", None) not in ("sem-inc", "sem-add-imm"):
                continue
            ticked = True
            lst = proc_events.setdefault(u.ant_name, [])
            prev = lst[-1][0] if lst else 0
            lst.append((prev + (u.update_value or 1), ins))
        waits_of[id(ins)] = all_waits if ticked else ow
        pending[eng] = [] if ticked else all_waits

    import bisect

    def prefix_index(sem, v):
        lst = proc_events.get(sem)
        if not lst:
            return None
        ticks = [t for t, _ in lst]
        i = bisect.bisect_left(ticks, v)
        return i if i < len(lst) else None

    memo = {}

    def holds(sem, v, depth=0):
        if depth > 6:
            return {}
        i = prefix_index(sem, v)
        if i is None:
            return {}
        key = (sem, i)
        if key in memo:
            return memo[key]
        memo[key] = {}
        out = {}
        inorder = not sem.startswith("Pool")
        rng = range(i + 1) if inorder else (i,)
        for j in rng:
            _, ins = proc_events[sem][j]
            for (s2, v2) in waits_of.get(id(ins), []):
                if out.get(s2, 0) < v2:
                    out[s2] = v2
                sub = holds(s2, v2, depth + 1)
                for s3, v3 in sub.items():
                    if out.get(s3, 0) < v3:
                        out[s3] = v3
        memo[key] = out
        return out

    own_tick = {}
    for sem, lst in proc_events.items():
        for tick, ins in lst:
            own_tick[(id(ins), sem)] = tick

    pruned = 0
    for ins in insts:
        si = getattr(ins, "sync_info", None)
        if si is None or not si.on_wait or len(si.on_wait) < 2:
            continue
        ow = list(si.on_wait)
        kept = list(ow)
        tn = type(ins).__name__
        is_dma = "DMA" in tn or "Drain" in tn
        for w in ow:
            if len(kept) == 1:
                break
            # same-queue FIFO rule, DMA instructions only: waiting on earlier
            # completions of the queue this DMA executes on is vacuous
            # (per-queue serial execution). Compute engines keep such waits:
            # the race detector requires them when APs overlap.
            if is_dma:
                mine = own_tick.get((id(ins), w.ant_name))
                if mine is not None and w.wait_value <= mine - 1:
                    kept.remove(w)
                    pruned += 1
                    continue
            others = [o for o in kept if o is not w]
            for o in others:
                h = holds(o.ant_name, o.wait_value)
                if h.get(w.ant_name, 0) >= w.wait_value:
                    kept.remove(w)
                    pruned += 1
                    break
        si.on_wait = kept
    return pruned


def _host_prep(x, conv_w, centroids):
    from concourse import mybir
    bf16np = mybir.dt.np(mybir.dt.bfloat16)
    fp8np = mybir.dt.np(mybir.dt.float8e4)

    x = np.ascontiguousarray(x, dtype=np.float32)
    norm = np.sqrt((x.astype(np.float64) ** 2).sum(0))
    xn = (x / np.maximum(norm, 1e-12)).astype(np.float32)    # [C,H,W]
    ii = np.arange(H, dtype=np.float64)
    mi = np.minimum(H - 1 - ii, ii)
    m = np.minimum(mi[:, None], mi[None, :])
    m4 = m ** 4
    # rescale so w2 = msk*soft*cnt fits fp8e4m3 range; the global scale
    # cancels in the intra-cluster L2 normalization on the host
    msk_full = (m4 / m4.max()).astype(np.float32)            # [H,W]

    xn_pad = np.zeros((C, H + 2, W), np.float32)
    xn_pad[:, 1:H + 1] = xn
    msk_pad = np.zeros((H + 2, W), np.float32)
    msk_pad[1:H + 1] = msk_full

    # packed small inputs
    cwtb = conv_w.T.astype(np.float32).reshape(CT, 128, K)
    cwtb = np.ascontiguousarray(cwtb.transpose(1, 0, 2)).reshape(128, CT * K)
    small8 = np.zeros((128, CT * K + 8), np.float32)
    small8[:, 0:CT * K] = cwtb
    small8[:, CT * K:] = 1.0
    small8 = small8.astype(fp8np)
    identb = np.eye(128, dtype=np.float32)
    mstack = np.concatenate([np.eye(K), np.eye(K)], 0).astype(np.float32)
    slot = np.array(SLOT_TILES)

    in_maps = []
    for core in range(M):
        r0 = core * RPC
        slab = np.ascontiguousarray(
            xn_pad[:, r0:r0 + RPC + 2, :].reshape(C, Ls))
        mskc = msk_pad[r0:r0 + RPC + 2].reshape(Ls).copy()
        mskc[0:W] = 0.0
        mskc[(RPC + 1) * W:] = 0.0                           # halo rows -> 0
        xnT = np.ascontiguousarray(slab.T).astype(fp8np)     # [Ls, C]
        # permute l-tiles into VLAD slot order
        xnT_perm = np.ascontiguousarray(
            xnT.reshape(NT, 128, C)[slot].reshape(Ls, C))
        smallb = np.zeros((128, 128 + K + NT), np.float32)
        smallb[:, 0:128] = identb
        smallb[:, 128:128 + K] = mstack
        smallb[:, 128 + K:] = mskc.reshape(NT, 128).T
        in_maps.append({
            "xnb": slab.astype(fp8np),
            "xnt": xnT_perm,
            "smallb": smallb.astype(bf16np),
            "small8": small8,
        })
    return in_maps


def _ensure_ntff_hook():
    """Install the axon NTFF profile hook if the image's antenv lacks it."""
    import types
    try:
        from antenv.axon_hooks import get_axon_ntff_profile_hook  # noqa: F401
        return
    except ImportError:
        pass
    if "/root/.axon_site" not in sys.path:
        sys.path.insert(0, "/root/.axon_site")
    from trn_agent_boot.trn_boot import _ntff_profile_via_ctypes
    hook = _ntff_profile_via_ctypes("/opt/axon/libaxon_pjrt.so")
    mod = types.ModuleType("antenv.axon_hooks")
    mod.get_axon_ntff_profile_hook = lambda: hook
    mod.set_axon_ntff_profile_hook = lambda h: None
    import antenv
    antenv.axon_hooks = mod
    sys.modules["antenv.axon_hooks"] = mod


def _install_neff_cache():
    """Cache compiled NEFFs across processes, keyed by BIR content hash."""
    import hashlib
    import shutil
    import concourse.bass2jax as b2j

    orig = b2j.compile_bir_kernel
    if getattr(orig, "_neff_cached", False):
        return

    def cached(bir_json, tmpdir, neff_name="file.neff"):
        h = hashlib.sha256(
            bir_json if isinstance(bir_json, bytes) else bir_json.encode()
        ).hexdigest()[:24]
        cdir = "/tmp/neff_cache"
        os.makedirs(cdir, exist_ok=True)
        cpath = os.path.join(cdir, h + ".neff")
        if os.path.exists(cpath):
            dst = os.path.join(tmpdir, neff_name)
            os.makedirs(tmpdir, exist_ok=True)
            shutil.copy(cpath, dst)
            return dst
        out = orig(bir_json, tmpdir, neff_name=neff_name)
        shutil.copy(out, cpath)
        return out

    cached._neff_cached = True
    b2j.compile_bir_kernel = cached


def kernel(x, conv_w, centroids):
    import concourse.bass_utils as bu
    from concourse.bass_utils import run_bass_kernel_spmd
    _install_neff_cache()
    if TRACE:
        _ensure_ntff_hook()
        bu.upload_artifacts = lambda tmpdir: "local://" + tmpdir

    if "nc" not in _CACHE:
        _CACHE["nc"] = _build_nc()
    nc = _CACHE["nc"]
    in_maps = _host_prep(np.asarray(x), np.asarray(conv_w), np.asarray(centroids))
    res = run_bass_kernel_spmd(nc, in_maps, list(range(M)), trace=TRACE)
    _CACHE["last"] = res
    red = np.zeros((128, C + 1), np.float64)
    for r in res.results:
        red += np.asarray(r["y"], dtype=np.float64)
    redk = red[0:64] + red[64:128]                           # [K, C+1]
    vlad = redk[:, :C] - redk[:, C:C + 1] * np.asarray(centroids, np.float64)
    vlad /= np.maximum(np.sqrt((vlad ** 2).sum(1))[:, None], 1e-12)
    v = vlad.reshape(1, K * C)
    v /= np.maximum(np.sqrt((v ** 2).sum()), 1e-12)
    return v.astype(np.float32)
